# revision 1
# baseline (speedup 1.0000x reference)
"""Distributed multi-head attention kernel for 8 TRN2 NeuronCores.

Problem: x [4, 2048, 1024] -> qkv proj -> 16-head attention (d=64)
         -> out proj + bias -> [4, 2048, 1024].

Sharding (no collectives): core i handles batch b = i//2 and query-half
half = i%2 (1024 query tokens). Each core computes K/V for its batch's
full 2048-token sequence (duplicated within the pair of cores sharing a
batch) and Q only for its own 1024 tokens. The host rotates the token
axis per core so the core's query tokens are always tokens [0, 1024) of
its input -- attention is permutation-invariant over keys, so K/V token
order does not matter.

Per-core pipeline (everything bf16 on the TensorE, fp32 PSUM accum):
  proj:  Q^T [d, q] / K^T [d, k] head-pairs packed on 128 partitions;
         V [k, d] in 65-wide per-head blocks with a ones column
         (the PV matmul then yields softmax denominators for free).
  attn:  per head: S^T = K @ Q^T -> exp on ScalarE (x0.125 fused, no
         max subtraction; scores are O(1) by construction) -> bf16 P^T
         -> PV accumulation U^T[65, q]; row 64 = denominator.
         Tail: U^T -> SBUF bf16 + 1/D (fp16) immediately (frees PSUM);
         normalize = K=1 ones matmul broadcast + DVE multiply, off the
         critical path.
  out:   two passes (heads 0-7 + bias, then heads 8-15) accumulating
         through a DRAM scratch so pass A fills the PE during the
         ACT-bound attention of the second half.

The two halves' projections and attention phases are arranged so the
PE always has matmul work while the ScalarE grinds through exp()
(keeps the PE HAM clock gate at 2.4 GHz).
"""

import numpy as np
import ml_dtypes

B = 4
N = 2048
DIM = 1024
HEADS = 16
DH = 64
NQ = 1024  # query tokens per core
NCORES = 8

_CACHE = {}


def _build_nc():
    from contextlib import ExitStack

    import concourse.bass as bass
    import concourse.mybir as mybir
    import concourse.tile as tile
    from concourse import bacc

    f32 = mybir.dt.float32
    bf16 = mybir.dt.bfloat16
    f16 = mybir.dt.float16
    EXP = mybir.ActivationFunctionType.Exp

    nc = bacc.Bacc("TRN2", target_bir_lowering=False, debug=False,
                   num_devices=NCORES)

    xt_d = nc.dram_tensor("xt", [DIM, N], bf16, kind="ExternalInput")
    wqkv_d = nc.dram_tensor("wqkv", [DIM, 3 * DIM], bf16, kind="ExternalInput")
    wo_d = nc.dram_tensor("wo", [HEADS, DH, DIM], bf16, kind="ExternalInput")
    brow_d = nc.dram_tensor("brow", [1, DIM], bf16, kind="ExternalInput")
    out_d = nc.dram_tensor("out", [NQ, DIM], f32, kind="ExternalOutput")

    with tile.TileContext(nc) as tc, ExitStack() as top:
        const_pool = top.enter_context(tc.tile_pool(name="const", bufs=1))
        mm_psum = top.enter_context(tc.tile_pool(name="mmps", bufs=2, space="PSUM"))
        sp_psum = top.enter_context(tc.tile_pool(name="spps", bufs=2, space="PSUM"))
        u_psum = top.enter_context(tc.tile_pool(name="ups", bufs=1, space="PSUM"))
        es_pool = top.enter_context(tc.tile_pool(name="es", bufs=6))
        rec_pool = top.enter_context(tc.tile_pool(name="rec", bufs=4))
        bc_pool = top.enter_context(tc.tile_pool(name="bc", bufs=3))
        uraw_a = top.enter_context(tc.tile_pool(name="uraw_a", bufs=1))
        dram_pool = top.enter_context(tc.tile_pool(name="dscr", bufs=1, space="DRAM"))

        brow_t = const_pool.tile([1, DIM], bf16, tag="brow", name="brow")
        nc.sync.dma_start(brow_t[:], brow_d.ap()[:])
        ones_t = const_pool.tile([1, 128], bf16, tag="ones", name="ones")
        nc.gpsimd.memset(ones_t[:], 1.0)
        ones_bc = const_pool.tile([128, 64], f16, tag="ones_bc", name="ones_bc")
        nc.gpsimd.memset(ones_bc[:], 1.0)

        uraw = [None] * HEADS

        def proj_units(half, w_pool, xt, QT, KT, VT):
            """Emission closures, one PSUM-group each.

            Order: all of V, then K/Q alternating per head-pair chunk so
            early head pairs become ready as soon as possible.
            """
            def dma_factory(col0):
                box = [None]
                def dma():
                    if box[0] is None:
                        wb = [w_pool.tile([128, 512], bf16, tag=f"w{fc}",
                                          name=f"w{fc}") for fc in range(8)]
                        for fc in range(8):
                            nc.sync.dma_start(
                                wb[fc][:],
                                wqkv_d.ap()[fc * 128:(fc + 1) * 128,
                                            col0:col0 + 512])
                        box[0] = wb
                    return box[0]
                return dma

            dma_v = dma_factory(2 * DIM + half * 512)
            dma_k = dma_factory(DIM + half * 512)
            dma_q = dma_factory(half * 512)

            def v_unit(mk, dma=dma_v):
                wb = dma()
                ps = mm_psum.tile([128, 512], f32, tag="mm", name="mm")
                for fc in range(8):
                    nc.tensor.matmul(
                        ps[:], xt[fc][:, mk * 128:(mk + 1) * 128], wb[fc][:],
                        start=(fc == 0), stop=(fc == 7))
                nc.vector.tensor_copy(
                    VT[mk][:, :, 0:64],
                    ps[:].rearrange("p (h d) -> p h d", d=64))
                nc.gpsimd.memset(VT[mk][:, :, 64:65], 1.0)

            def qk_unit(dma, dest, m4, t):
                wb = dma()
                ps = mm_psum.tile([128, 512], f32, tag="mm", name="mm")
                for fc in range(8):
                    nc.tensor.matmul(
                        ps[:], wb[fc][:, m4 * 128:(m4 + 1) * 128],
                        xt[fc][:, t * 512:(t + 1) * 512],
                        start=(fc == 0), stop=(fc == 7))
                nc.vector.tensor_copy(
                    dest[m4][:, t * 512:(t + 1) * 512], ps[:])

            units = [lambda mk=mk: v_unit(mk) for mk in range(16)]
            for m4 in range(4):
                for t in range(4):
                    units.append(lambda m4=m4, t=t: qk_unit(dma_k, KT, m4, t))
                for t in range(2):
                    units.append(lambda m4=m4, t=t: qk_unit(dma_q, QT, m4, t))
            return units

        def emit_norm(h):
            """Normalize head h's raw U by its softmax denominators."""
            rec = _recs.pop(h)
            for qc in range(2):
                bc = mm_psum.tile([64, 512], f32, tag="mm", name="bc")
                nc.tensor.matmul(
                    bc[:], ones_bc[64:65, :],
                    rec[64:65, qc * 512:(qc + 1) * 512],
                    start=True, stop=True)
                bc_sb = bc_pool.tile([64, 512], f32, tag="bc", name="bc_sb")
                nc.vector.tensor_copy(bc_sb[:], bc[:])
                nc.gpsimd.tensor_mul(
                    uraw[h][0:64, qc * 512:(qc + 1) * 512],
                    uraw[h][0:64, qc * 512:(qc + 1) * 512], bc_sb[:])

        _recs = {}

        def emit_attn(heads, QTs, KTs, VTs, uraw_pools, fillers,
                      hooks=None):
            """Attention for the given heads; filler units spread across
            k-steps. Normalize for head h is emitted one head late."""
            fillers = list(fillers)
            nfill = len(fillers)
            steps = len(heads) * 16
            done = 0
            for hh_i, h in enumerate(heads):
                if hooks and h in hooks:
                    hooks[h]()
                half = h // 8
                hh = h % 8
                QT, KT, VT = QTs[half], KTs[half], VTs[half]
                pair = hh // 2
                hb = (hh % 2) * 64
                Ups = u_psum.tile([65, 2, 512], f32, tag="up", name="up")
                for k in range(16):
                    sp = sp_psum.tile([128, 2, 512], f32, tag="sp", name="sp")
                    for qc in range(2):
                        nc.tensor.matmul(
                            sp[:, qc, :],
                            KT[pair][hb:hb + 64, k * 128:(k + 1) * 128],
                            QT[pair][hb:hb + 64, qc * 512:(qc + 1) * 512],
                            start=True, stop=True)
                    es = es_pool.tile([128, 2, 512], bf16, tag="es", name="es")
                    nc.scalar.activation(es[:], sp[:], EXP, scale=0.125)
                    for qc in range(2):
                        nc.tensor.matmul(
                            Ups[:, qc, :],
                            VT[k][:, hh, :],
                            es[:, qc, :],
                            start=(k == 0), stop=(k == 15))
                    done += 1
                    while fillers and (nfill - len(fillers)) * steps < done * nfill:
                        fillers.pop(0)()
                # free the PSUM slot fast: one copy stashes raw U + D
                ur = uraw_pools[half]().tile([65, NQ], bf16, tag=f"uraw{h}",
                                             name=f"uraw{h}")
                uraw[h] = ur
                nc.vector.tensor_copy(
                    ur[:].rearrange("p (a b) -> p a b", a=2), Ups[:, :, :])
                # slow reciprocal runs from SBUF, off the PSUM critical chain
                rec = rec_pool.tile([65, NQ], f16, tag="rec", name="rec")
                with nc.allow_low_precision(reason="softmax denom recip fp16"):
                    nc.vector.reciprocal(
                        rec[64:65, :], ur[64:65, :])
                _recs[h] = rec
                if hh_i > 0:
                    emit_norm(heads[hh_i - 1])
            emit_norm(heads[-1])
            for f in fillers:
                f()

        # ---------------- emission ----------------
        if True:
            xt_pool = tc.alloc_tile_pool(name="xt", bufs=1)
            w_pool = tc.alloc_tile_pool(name="w", bufs=2)
            xt = [xt_pool.tile([128, N], bf16, tag=f"xt{i}", name=f"xt{i}")
                  for i in range(8)]
            for i in range(8):
                nc.sync.dma_start(xt[i][:], xt_d.ap()[i * 128:(i + 1) * 128, :])

            qkv0 = tc.alloc_tile_pool(name="qkv0", bufs=1)
            QT0 = [qkv0.tile([128, NQ], bf16, tag=f"q{m}", name=f"q0{m}")
                   for m in range(4)]
            KT0 = [qkv0.tile([128, N], bf16, tag=f"k{m}", name=f"k0{m}")
                   for m in range(4)]
            VT0 = [qkv0.tile([128, 8, 65], bf16, tag=f"v{mk}", name=f"v0{mk}")
                   for mk in range(16)]
            p0_units = proj_units(0, w_pool, xt, QT0, KT0, VT0)
            # V + pair-0 K/Q serially (heads 0/1 cannot start without them;
            # Tile dependencies only look backward in emission order)
            for c in p0_units[:22]:
                c()
            p0_rest = p0_units[22:]

            qkv1 = tc.alloc_tile_pool(name="qkv1", bufs=1, side="right")
            QT1 = [qkv1.tile([128, NQ], bf16, tag=f"q{m}", name=f"q1{m}")
                   for m in range(4)]
            KT1 = [qkv1.tile([128, N], bf16, tag=f"k{m}", name=f"k1{m}")
                   for m in range(4)]
            VT1 = [qkv1.tile([128, 8, 65], bf16, tag=f"v{mk}", name=f"v1{mk}")
                   for mk in range(16)]
            p1_units = proj_units(1, w_pool, xt, QT1, KT1, VT1)

            state = {}

            def setup_b():
                qkv0.release()
                state["uraw_b"] = tc.alloc_tile_pool(name="uraw_b", bufs=1,
                                                     side="right")
                wo_pool = tc.alloc_tile_pool(name="wo", bufs=1, side="right")
                state["wo_pool"] = wo_pool
                state["WO"] = [wo_pool.tile([64, DIM], bf16, tag=f"wo{h}",
                                            name=f"wo{h}")
                               for h in range(HEADS)]
                for h in range(HEADS):
                    nc.sync.dma_start(state["WO"][h][:], wo_d.ap()[h])


            def setup_c():
                # xt and the w-block tiles are dead once proj(1) is done
                w_pool.release()
                xt_pool.release()
                state["st_pool"] = tc.alloc_tile_pool(name="st", bufs=2)
                state["FIN"] = [
                    state["st_pool"].tile([128, DIM], f32, tag=f"fin{qf}",
                                          name=f"fin{qf}", bufs=1)
                    for qf in range(8)]

            # pass A unit: heads 0-7 + bias for one qf -> resident FIN tile
            def passA(qf):
                WO = state["WO"]
                fin = state["FIN"][qf]
                for of in range(2):
                    ps = mm_psum.tile([128, 512], f32, tag="mm", name="mm")
                    for hh in range(8):
                        nc.tensor.matmul(
                            ps[:],
                            uraw[hh][0:64, qf * 128:(qf + 1) * 128],
                            WO[hh][:, of * 512:(of + 1) * 512],
                            start=(hh == 0), stop=False)
                    nc.tensor.matmul(
                        ps[:], ones_t[:, 0:128],
                        brow_t[:, of * 512:(of + 1) * 512],
                        start=False, stop=True)
                    nc.vector.tensor_copy(fin[:, of * 512:(of + 1) * 512],
                                          ps[:])

            uraw_pools = {0: lambda: uraw_a, 1: lambda: state["uraw_b"]}

            # heads 0-9: remaining proj0 + all proj1 units fill PE gaps
            emit_attn(range(0, 10), [QT0, QT1], [KT0, KT1], [VT0, VT1],
                      uraw_pools, p0_rest + p1_units[:34],
                      hooks={8: setup_b})
            # heads 10-15: pass A units fill PE gaps
            emit_attn(range(10, 16), [QT0, QT1], [KT0, KT1], [VT0, VT1],
                      uraw_pools,
                      p1_units[34:] +
                      [lambda qf=qf: passA(qf) for qf in range(8)],
                      hooks={10: setup_c})

            # pass B: heads 8-15 onto the resident partials -> out.
            # h15 (the norm-gated head) goes FIRST in each accumulation
            # group so the PE pays its normalize wait once, then streams.
            for qf in range(8):
                fin = state["FIN"][qf]
                for of in range(2):
                    ps = mm_psum.tile([128, 512], f32, tag="mm", name="mm")
                    for hh in [15] + list(range(8, 15)):
                        nc.tensor.matmul(
                            ps[:],
                            uraw[hh][0:64, qf * 128:(qf + 1) * 128],
                            state["WO"][hh][:, of * 512:(of + 1) * 512],
                            start=(hh == 15), stop=(hh == 14))
                    nc.vector.tensor_add(
                        fin[:, of * 512:(of + 1) * 512],
                        fin[:, of * 512:(of + 1) * 512], ps[:])
                nc.sync.dma_start(out_d.ap()[qf * 128:(qf + 1) * 128, :], fin[:])

            state["st_pool"].release()
            state["wo_pool"].release()
            state["uraw_b"].release()
            qkv1.release()

    nc.compile()
    return nc


def _get_nc():
    if "nc" not in _CACHE:
        _CACHE["nc"] = _build_nc()
    return _CACHE["nc"]


def _make_in_maps(x, w_qkv, w_out, b_out):
    bf = ml_dtypes.bfloat16
    wo = np.ascontiguousarray(w_out.reshape(HEADS, DH, DIM)).astype(bf)
    brow = np.asarray(b_out, np.float32).reshape(1, DIM).astype(bf)
    wqkv = np.ascontiguousarray(w_qkv, np.float32).astype(bf)
    in_maps = []
    for i in range(NCORES):
        b, half = i // 2, i % 2
        xt = np.asarray(x[b], np.float32).T.astype(bf)  # [DIM, N]
        if half:
            xt = np.concatenate([xt[:, NQ:], xt[:, :NQ]], axis=1)
        in_maps.append({
            "xt": np.ascontiguousarray(xt),
            "wqkv": wqkv,
            "wo": wo,
            "brow": brow,
        })
    return in_maps


def _assemble(results):
    out = np.empty((B, N, DIM), np.float32)
    for i in range(NCORES):
        b, half = i // 2, i % 2
        out[b, half * NQ:(half + 1) * NQ, :] = results[i]["out"]
    return out


def run(x, w_qkv, w_out, b_out, trace=False):
    """Run the kernel; returns (output, BassKernelResults)."""
    from concourse.bass_utils import run_bass_kernel_spmd
    nc = _get_nc()
    in_maps = _make_in_maps(x, w_qkv, w_out, b_out)
    res = run_bass_kernel_spmd(nc, in_maps, core_ids=list(range(NCORES)),
                               trace=trace)
    return _assemble(res.results), res


def kernel(x, w_qkv, w_out, b_out):
    out, _ = run(x, w_qkv, w_out, b_out, trace=False)
    return out



# revision 10
# speedup vs baseline: 1.1584x; 1.1584x over previous
"""Distributed multi-head attention kernel for 8 TRN2 NeuronCores.

Problem: x [4, 2048, 1024] -> qkv proj -> 16-head attention (d=64)
         -> out proj + bias -> [4, 2048, 1024].

Sharding (head-split, no collectives): core i handles batch b = i//2 and
head-half hh = i%2 (8 heads, full 2048-token sequence). Each core
computes Q/K/V projections only for its own 8 heads (no duplicated
work), attention for those heads, and a partial output projection
out_partial = sum_h (U_h/D_h) @ Wo_h (+bias on hh=0 cores). The host
sums the two partial outputs per batch -- host-side adds are free.

Per-core pipeline, built around the ScalarE exp() bottleneck
(~33.5M exp/core; ACTIVATE costs (N+352)/1.2 ns per instruction, so we
use [128,1024] psum spans, double-buffered, and keep ACT 100% busy):

  proj:  Q^T/K^T [128 outdims = head-pair, tok] tiles; V [tok, 8, 65]
         with a ones column (PV then yields softmax denominators free).
  attn:  per (head, q-chunk of 1024): for kc in 16: S^T [128,1024] psum
         (2 MMs) -> exp on ScalarE -> es bf16 -> PV accumulate
         U [65,1024] psum. Software-pipelined: PE emits S two steps
         ahead so ACT never gaps; U is drained to SBUF immediately
         (raw + D row) to free the single U psum buffer; the divide
         (K=1 broadcast matmul of 1/D + gpsimd multiply into the
         pair-packed UN tile) runs lazily as filler work.
  out:   pass A (pairs 0-2 + bias -> resident FIN tiles) as fillers
         during pair-3 attention; pass B (pair 3) + DMA as the tail.
"""

import numpy as np
import ml_dtypes

B = 4
N = 2048
DIM = 1024
HEADS = 16
DH = 64
NCORES = 8
NH = 8       # heads per core
NPAIR = 4    # head pairs per core

_CACHE = {}


def _build_nc():
    from contextlib import ExitStack

    import concourse.bass as bass
    import concourse.mybir as mybir
    import concourse.tile as tile
    from concourse import bacc

    f32 = mybir.dt.float32
    bf16 = mybir.dt.bfloat16
    f16 = mybir.dt.float16
    EXP = mybir.ActivationFunctionType.Exp

    nc = bacc.Bacc("TRN2", target_bir_lowering=False, debug=False,
                   num_devices=NCORES)

    xt_d = nc.dram_tensor("xt", [DIM, N], bf16, kind="ExternalInput")
    wq_d = nc.dram_tensor("wq", [DIM, 512], bf16, kind="ExternalInput")
    wk_d = nc.dram_tensor("wk", [DIM, 512], bf16, kind="ExternalInput")
    wv_d = nc.dram_tensor("wv", [DIM, 512], bf16, kind="ExternalInput")
    wo_d = nc.dram_tensor("wo", [NPAIR, 128, DIM], bf16, kind="ExternalInput")
    bias_d = nc.dram_tensor("bias", [128, DIM], bf16, kind="ExternalInput")
    out_d = nc.dram_tensor("out", [N, DIM], f32, kind="ExternalOutput")

    with tile.TileContext(nc) as tc, ExitStack() as top:
        const_pool = top.enter_context(tc.tile_pool(name="const", bufs=1))
        s_ps = top.enter_context(tc.tile_pool(name="sps", bufs=2, space="PSUM"))
        u_ps = top.enter_context(tc.tile_pool(name="ups", bufs=1, space="PSUM"))
        mm_ps = top.enter_context(tc.tile_pool(name="mmps", bufs=2, space="PSUM"))
        es_pool = top.enter_context(tc.tile_pool(name="es", bufs=3))
        ur_pool = top.enter_context(tc.tile_pool(name="ur", bufs=4))
        d_pool = top.enter_context(tc.tile_pool(name="dsb", bufs=4))
        r_pool = top.enter_context(tc.tile_pool(name="rsb", bufs=2))
        un_pool = top.enter_context(tc.tile_pool(name="un", bufs=1))

        bias_t = const_pool.tile([128, DIM], bf16, tag="bias", name="bias")
        nc.sync.dma_start(bias_t[:], bias_d.ap()[:])
        ones_t = const_pool.tile([1, 128], f16, tag="ones", name="ones")
        nc.gpsimd.memset(ones_t[:], 1.0)

        # ---- static input tiles -------------------------------------
        # (xt/w innermost: released mid-kernel; pool releases are LIFO)
        qkv_pool = tc.alloc_tile_pool(name="qkv", bufs=1)
        wo_pool = tc.alloc_tile_pool(name="wo", bufs=1)
        xt_pool = tc.alloc_tile_pool(name="xt", bufs=1)
        w_pool = tc.alloc_tile_pool(name="w", bufs=1)
        xt = [xt_pool.tile([128, N], bf16, tag=f"xt{i}", name=f"xt{i}")
              for i in range(8)]
        wq = [w_pool.tile([128, 512], bf16, tag=f"wq{i}", name=f"wq{i}")
              for i in range(8)]
        wk = [w_pool.tile([128, 512], bf16, tag=f"wk{i}", name=f"wk{i}")
              for i in range(8)]
        wv = [w_pool.tile([128, 512], bf16, tag=f"wv{i}", name=f"wv{i}")
              for i in range(8)]
        WO = [wo_pool.tile([128, DIM], bf16, tag=f"wo{p}", name=f"wo{p}")
              for p in range(NPAIR)]
        for i in range(8):
            nc.sync.dma_start(wq[i][:], wq_d.ap()[i * 128:(i + 1) * 128, :])
            nc.sync.dma_start(wk[i][:], wk_d.ap()[i * 128:(i + 1) * 128, :])
            nc.sync.dma_start(wv[i][:], wv_d.ap()[i * 128:(i + 1) * 128, :])
            nc.sync.dma_start(xt[i][:], xt_d.ap()[i * 128:(i + 1) * 128, :])
        for p in range(NPAIR):
            nc.sync.dma_start(WO[p][:], wo_d.ap()[p])

        QT = [qkv_pool.tile([128, N], bf16, tag=f"q{p}", name=f"q{p}")
              for p in range(NPAIR)]
        KT = [qkv_pool.tile([128, N], bf16, tag=f"k{p}", name=f"k{p}")
              for p in range(NPAIR)]
        VT = [qkv_pool.tile([128, NH, 65], bf16, tag=f"v{tb}", name=f"v{tb}")
              for tb in range(16)]
        UN = [un_pool.tile([128, N], bf16, tag=f"un{p}", name=f"un{p}")
              for p in range(NPAIR)]

        # ---- projection unit closures -------------------------------
        def kq_unit(dest, w, p, t):
            ps = mm_ps.tile([128, 512], f32, tag="mm", name="mm")
            for fc in range(8):
                nc.tensor.matmul(
                    ps[:], w[fc][:, p * 128:(p + 1) * 128],
                    xt[fc][:, t * 512:(t + 1) * 512],
                    start=(fc == 0), stop=(fc == 7))
            nc.vector.tensor_copy(dest[p][:, t * 512:(t + 1) * 512], ps[:])

        def v_unit(tb):
            ps = mm_ps.tile([128, 512], f32, tag="mm", name="mm")
            for fc in range(8):
                nc.tensor.matmul(
                    ps[:], xt[fc][:, tb * 128:(tb + 1) * 128], wv[fc][:],
                    start=(fc == 0), stop=(fc == 7))
            nc.vector.tensor_copy(
                VT[tb][:, :, 0:64],
                ps[:].rearrange("p (h d) -> p h d", d=64))
            nc.gpsimd.memset(VT[tb][:, :, 64:65], 1.0)

        # ---- fillers with deadlines ---------------------------------
        # each entry: (latest_gidx, closure); popped proportionally or
        # when the deadline is hit.
        fillers = []

        def add_filler(latest, fn):
            fillers.append((latest, fn))

        state = {"emitted": 0, "total": 0}

        def maybe_fill(done, steps):
            # pop through the last due entry (deadlines force front-running),
            # then keep pace proportionally.
            due_idx = -1
            for i, (latest, _) in enumerate(fillers):
                if latest is not None and done >= latest:
                    due_idx = i
            while due_idx >= 0:
                _, fn = fillers.pop(0)
                fn()
                state["emitted"] += 1
                due_idx -= 1
            while fillers:
                latest, fn = fillers[0]
                if state["emitted"] * steps >= done * state["total"]:
                    break
                fillers.pop(0)
                fn()
                state["emitted"] += 1

        # preamble: what head 0 / qc 0 needs up front
        kq_unit(KT, wk, 0, 0)
        kq_unit(KT, wk, 0, 1)
        kq_unit(QT, wq, 0, 0)
        kq_unit(QT, wq, 0, 1)
        for tb in range(6):
            v_unit(tb)

        # remaining proj work as deadline fillers. Deadlines are EMISSION
        # deadlines: S(g) is emitted at g-2, so a tile read at gidx g must
        # be written by done <= g-4 (margin for the in-body emission order).
        for tb in range(6, 16):
            add_filler(max(0, tb - 4), lambda tb=tb: v_unit(tb))
        for t in range(2, 4):
            add_filler(4 * t - 6, lambda t=t: kq_unit(KT, wk, 0, t))
        for t in range(2, 4):
            add_filler(12, lambda t=t: kq_unit(QT, wq, 0, t))
        for p in range(1, NPAIR):
            for t in range(4):
                add_filler(64 * p + 4 * t - 12,
                           lambda p=p, t=t: kq_unit(KT, wk, p, t))
            for t in range(4):
                add_filler(64 * p + 32 * (t // 2) - 12,
                           lambda p=p, t=t: kq_unit(QT, wq, p, t))

        # ---- attention, software-pipelined across all 16 units ------
        units = [(p, hh, qc) for p in range(NPAIR) for hh in range(2)
                 for qc in range(2)]
        NU = len(units)
        GTOT = NU * 16

        def emit_S(gidx):
            u, kc = divmod(gidx, 16)
            p, hh, qc = units[u]
            hb = hh * 64
            st = s_ps.tile([128, 1024], f32, tag="s", name="s")
            for j in range(2):
                nc.tensor.matmul(
                    st[:, j * 512:(j + 1) * 512],
                    KT[p][hb:hb + 64, kc * 128:(kc + 1) * 128],
                    QT[p][hb:hb + 64,
                          qc * 1024 + j * 512:qc * 1024 + j * 512 + 512],
                    start=True, stop=True)
            return st

        def norm_rest(p, hh, qc, ur, dsb):
            """Lazy normalize: broadcast 1/D and multiply into UN."""
            hb = hh * 64
            for j in range(2):
                bc = mm_ps.tile([128, 512], f32, tag="mm", name="bc")
                nc.tensor.matmul(bc[:], ones_t[:],
                                 dsb[:, j * 512:(j + 1) * 512],
                                 start=True, stop=True)
                rs = r_pool.tile([64, 512], f32, tag="rs", name="rs")
                nc.vector.reciprocal_approx_fast(rs[:], bc[0:64, :])
                nc.gpsimd.tensor_mul(
                    UN[p][hb:hb + 64,
                          qc * 1024 + j * 512:qc * 1024 + j * 512 + 512],
                    ur[:, j * 512:(j + 1) * 512], rs[:])

        S_tiles = {0: emit_S(0), 1: emit_S(1)}
        U_box = [None]

        passA_added = [False]
        fin_state = {}

        def setup_fin():
            w_pool.release()
            xt_pool.release()
            fin_state["pool"] = tc.alloc_tile_pool(name="fin", bufs=1)
            fin_state["FIN"] = [
                fin_state["pool"].tile([128, DIM], f32, tag=f"fin{qf}",
                                       name=f"fin{qf}")
                for qf in range(16)]

        def passA(qf):
            FIN = fin_state["FIN"]
            for of in range(2):
                ps = mm_ps.tile([128, 512], f32, tag="mm", name="pa")
                for p in range(3):
                    nc.tensor.matmul(
                        ps[:], UN[p][:, qf * 128:(qf + 1) * 128],
                        WO[p][:, of * 512:(of + 1) * 512],
                        start=(p == 0), stop=(p == 2))
                nc.vector.tensor_add(
                    FIN[qf][:, of * 512:(of + 1) * 512], ps[:],
                    bias_t[:, of * 512:(of + 1) * 512])

        for gidx in range(GTOT):
            u, kc = divmod(gidx, 16)
            p, hh, qc = units[u]
            hloc = 2 * p + hh
            st = S_tiles.pop(gidx)
            es = es_pool.tile([128, 1024], bf16, tag="es", name="es")
            nc.scalar.activation(es[:], st[:], EXP, scale=0.125)
            if kc == 0:
                U_box[0] = u_ps.tile([65, 1024], f32, tag="u", name="u")
            U = U_box[0]
            for j in range(2):
                nc.tensor.matmul(
                    U[:, j * 512:(j + 1) * 512],
                    VT[kc][:, hloc, 0:65],
                    es[:, j * 512:(j + 1) * 512],
                    start=(kc == 0), stop=(kc == 15))
            if gidx + 2 < GTOT:
                S_tiles[gidx + 2] = emit_S(gidx + 2)
            if kc == 15:
                # fast U drain: free the single U psum buffer ASAP
                ur = ur_pool.tile([64, 1024], bf16, tag="ur", name="ur")
                nc.vector.tensor_copy(ur[:], U[0:64, :])
                dsb = d_pool.tile([1, 1024], f16, tag="d", name="d")
                with nc.allow_low_precision(reason="softmax denom f16"):
                    nc.vector.tensor_copy(dsb[:], U[64:65, :])
                add_filler(gidx + 24,
                           lambda p=p, hh=hh, qc=qc, ur=ur, dsb=dsb:
                           norm_rest(p, hh, qc, ur, dsb))
                if u == 11 and not passA_added[0]:
                    passA_added[0] = True
                    setup_fin()
                    for qf in range(16):
                        add_filler(None, lambda qf=qf: passA(qf))
            state["total"] = max(state["total"], state["emitted"] + len(fillers))
            maybe_fill(gidx, GTOT)

        # flush remaining fillers (incl. last norms and any passA)
        while fillers:
            _, fn = fillers.pop(0)
            fn()

        # pass B: pair 3 onto FIN, then DMA out
        FIN = fin_state["FIN"]
        for qf in range(16):
            for of in range(2):
                ps = mm_ps.tile([128, 512], f32, tag="mm", name="pb")
                nc.tensor.matmul(
                    ps[:], UN[3][:, qf * 128:(qf + 1) * 128],
                    WO[3][:, of * 512:(of + 1) * 512],
                    start=True, stop=True)
                nc.vector.tensor_add(
                    FIN[qf][:, of * 512:(of + 1) * 512],
                    FIN[qf][:, of * 512:(of + 1) * 512], ps[:])
            nc.sync.dma_start(out_d.ap()[qf * 128:(qf + 1) * 128, :],
                              FIN[qf][:])

        fin_state["pool"].release()
        wo_pool.release()
        qkv_pool.release()

    nc.compile()
    return nc


def _get_nc():
    if "nc" not in _CACHE:
        _CACHE["nc"] = _build_nc()
    return _CACHE["nc"]


def _make_in_maps(x, w_qkv, w_out, b_out):
    bf = ml_dtypes.bfloat16
    xts = [np.ascontiguousarray(np.asarray(x[b], np.float32).T).astype(bf)
           for b in range(B)]
    wq_f = np.asarray(w_qkv[:, 0:1024], np.float32)
    wk_f = np.asarray(w_qkv[:, 1024:2048], np.float32)
    wv_f = np.asarray(w_qkv[:, 2048:3072], np.float32)
    wo_f = np.asarray(w_out, np.float32)  # [1024 inner, 1024 out]
    bias_rep = np.broadcast_to(
        np.asarray(b_out, np.float32).reshape(1, DIM), (128, DIM))
    zeros = np.zeros((128, DIM), np.float32)
    in_maps = []
    for i in range(NCORES):
        b, hh = i // 2, i % 2
        cs = slice(hh * 512, (hh + 1) * 512)
        wo_core = np.ascontiguousarray(
            wo_f[hh * 512:(hh + 1) * 512, :]).reshape(NPAIR, 128, DIM)
        in_maps.append({
            "xt": xts[b],
            "wq": np.ascontiguousarray(wq_f[:, cs]).astype(bf),
            "wk": np.ascontiguousarray(wk_f[:, cs]).astype(bf),
            "wv": np.ascontiguousarray(wv_f[:, cs]).astype(bf),
            "wo": wo_core.astype(bf),
            "bias": np.ascontiguousarray(
                (bias_rep if hh == 0 else zeros)).astype(bf),
        })
    return in_maps


def _assemble(results):
    out = np.empty((B, N, DIM), np.float32)
    for b in range(B):
        out[b] = results[2 * b]["out"] + results[2 * b + 1]["out"]
    return out


def run(x, w_qkv, w_out, b_out, trace=False):
    """Run the kernel; returns (output, BassKernelResults)."""
    from concourse.bass_utils import run_bass_kernel_spmd
    nc = _get_nc()
    in_maps = _make_in_maps(x, w_qkv, w_out, b_out)
    res = run_bass_kernel_spmd(nc, in_maps, core_ids=list(range(NCORES)),
                               trace=trace)
    return _assemble(res.results), res


def kernel(x, w_qkv, w_out, b_out):
    out, _ = run(x, w_qkv, w_out, b_out, trace=False)
    return out


# revision 17
# speedup vs baseline: 1.4361x; 1.2398x over previous
"""Distributed multi-head attention kernel for 8 TRN2 NeuronCores.

Problem: x [4, 2048, 1024] -> qkv proj -> 16-head attention (d=64)
         -> out proj + bias -> [4, 2048, 1024].

Sharding (head-split, no collectives): core i handles batch b = i//2 and
head-half hh = i%2 (8 heads, full 2048-token sequence). Each core
computes Q/K/V projections only for its own 8 heads (no duplicated
work), attention for those heads, and a partial output projection
out_partial = sum_h (U_h/D_h) @ Wo_h (+bias on hh=0 cores). The host
sums the two partial outputs per batch -- host-side adds are free.

Per-core pipeline, built around the ScalarE exp() bottleneck
(~33.5M exp/core; ACTIVATE costs (N+352)/1.2 ns per instruction, so we
use [128,1024] psum spans, double-buffered, and keep ACT 100% busy):

  proj:  Q^T/K^T [128 outdims = head-pair, tok] tiles; V [tok, 8, 65]
         with a ones column (PV then yields softmax denominators free).
  attn:  per (head, q-chunk of 1024): for kc in 16: S^T [128,1024] psum
         (2 MMs) -> exp on ScalarE -> es bf16 -> PV accumulate
         U [65,1024] psum. Software-pipelined: PE emits S two steps
         ahead so ACT never gaps; U is drained to SBUF immediately
         (raw + D row) to free the single U psum buffer; the divide
         (K=1 broadcast matmul of 1/D + gpsimd multiply into the
         pair-packed UN tile) runs lazily as filler work.
  out:   pass A (pairs 0-2 + bias -> resident FIN tiles) as fillers
         during pair-3 attention; pass B (pair 3) + DMA as the tail.
"""

import numpy as np
import ml_dtypes

B = 4
N = 2048
DIM = 1024
HEADS = 16
DH = 64
NCORES = 8
NH = 8       # heads per core
NPAIR = 4    # head pairs per core

_CACHE = {}


def _build_nc():
    from contextlib import ExitStack

    import concourse.bass as bass
    import concourse.mybir as mybir
    import concourse.tile as tile
    from concourse import bacc

    f32 = mybir.dt.float32
    bf16 = mybir.dt.bfloat16
    f16 = mybir.dt.float16
    EXP = mybir.ActivationFunctionType.Exp

    nc = bacc.Bacc("TRN2", target_bir_lowering=False, debug=False,
                   num_devices=NCORES)

    xt_d = nc.dram_tensor("xt", [DIM, N], bf16, kind="ExternalInput")
    wq_d = nc.dram_tensor("wq", [DIM, 512], bf16, kind="ExternalInput")
    wk_d = nc.dram_tensor("wk", [DIM, 512], bf16, kind="ExternalInput")
    wv_d = nc.dram_tensor("wv", [DIM, 512], bf16, kind="ExternalInput")
    wo_d = nc.dram_tensor("wo", [NPAIR, 128, DIM], bf16, kind="ExternalInput")
    bias_d = nc.dram_tensor("bias", [128, DIM], bf16, kind="ExternalInput")
    out_d = nc.dram_tensor("out", [N, DIM], f32, kind="ExternalOutput")

    with tile.TileContext(nc) as tc, ExitStack() as top:
        const_pool = top.enter_context(tc.tile_pool(name="const", bufs=1))
        s_ps = top.enter_context(tc.tile_pool(name="sps", bufs=2, space="PSUM"))
        u_ps = top.enter_context(tc.tile_pool(name="ups", bufs=1, space="PSUM"))
        mm_ps = top.enter_context(tc.tile_pool(name="mmps", bufs=2, space="PSUM"))
        es_pool = top.enter_context(tc.tile_pool(name="es", bufs=4))
        ur_pool = top.enter_context(tc.tile_pool(name="ur", bufs=4))
        d_pool = top.enter_context(tc.tile_pool(name="dsb", bufs=4))
        r_pool = top.enter_context(tc.tile_pool(name="rsb", bufs=2))
        un_pool = top.enter_context(tc.tile_pool(name="un", bufs=1))

        bias_t = const_pool.tile([128, DIM], bf16, tag="bias", name="bias")
        nc.sync.dma_start(bias_t[:], bias_d.ap()[:])
        ones_t = const_pool.tile([1, 128], f16, tag="ones", name="ones")
        nc.gpsimd.memset(ones_t[:], 1.0)

        # ---- static input tiles -------------------------------------
        # (xt/w innermost: released mid-kernel; pool releases are LIFO)
        qkv_pool = tc.alloc_tile_pool(name="qkv", bufs=1)
        wo_pool = tc.alloc_tile_pool(name="wo", bufs=1)
        xt_pool = tc.alloc_tile_pool(name="xt", bufs=1)
        w_pool = tc.alloc_tile_pool(name="w", bufs=1)
        xt = [xt_pool.tile([128, N], bf16, tag=f"xt{i}", name=f"xt{i}")
              for i in range(8)]
        wq = [w_pool.tile([128, 512], bf16, tag=f"wq{i}", name=f"wq{i}")
              for i in range(8)]
        wk = [w_pool.tile([128, 512], bf16, tag=f"wk{i}", name=f"wk{i}")
              for i in range(8)]
        wv = [w_pool.tile([128, 512], bf16, tag=f"wv{i}", name=f"wv{i}")
              for i in range(8)]
        WO = [wo_pool.tile([128, DIM], bf16, tag=f"wo{p}", name=f"wo{p}")
              for p in range(NPAIR)]
        for i in range(8):
            nc.sync.dma_start(wq[i][:], wq_d.ap()[i * 128:(i + 1) * 128, :])
            nc.sync.dma_start(wk[i][:], wk_d.ap()[i * 128:(i + 1) * 128, :])
            nc.sync.dma_start(wv[i][:], wv_d.ap()[i * 128:(i + 1) * 128, :])
            nc.sync.dma_start(xt[i][:], xt_d.ap()[i * 128:(i + 1) * 128, :])
        for p in range(NPAIR):
            nc.sync.dma_start(WO[p][:], wo_d.ap()[p])

        QT = [qkv_pool.tile([128, N], bf16, tag=f"q{p}", name=f"q{p}")
              for p in range(NPAIR)]
        KT = [qkv_pool.tile([128, N], bf16, tag=f"k{p}", name=f"k{p}")
              for p in range(NPAIR)]
        VT = [qkv_pool.tile([128, NH, 65], bf16, tag=f"v{tb}", name=f"v{tb}")
              for tb in range(16)]
        UN = [un_pool.tile([128, N], bf16, tag=f"un{p}", name=f"un{p}")
              for p in range(NPAIR)]

        # ---- projection unit closures -------------------------------
        # split in halves (4 MMs each) so a filler never inserts more
        # than ~900ns into the PE queue ahead of the ACT-critical chain.
        def kq_first(box, w, p, t):
            ps = mm_ps.tile([128, 512], f32, tag="mm", name="mm")
            box[0] = ps
            for fc in range(4):
                nc.tensor.matmul(
                    ps[:], w[fc][:, p * 128:(p + 1) * 128],
                    xt[fc][:, t * 512:(t + 1) * 512],
                    start=(fc == 0), stop=False)

        def kq_second(box, dest, w, p, t):
            ps = box[0]
            for fc in range(4, 8):
                nc.tensor.matmul(
                    ps[:], w[fc][:, p * 128:(p + 1) * 128],
                    xt[fc][:, t * 512:(t + 1) * 512],
                    start=False, stop=(fc == 7))
            nc.vector.tensor_copy(dest[p][:, t * 512:(t + 1) * 512], ps[:])

        def kq_unit(dest, w, p, t):
            box = [None]
            kq_first(box, w, p, t)
            kq_second(box, dest, w, p, t)

        def v_first(box, tb):
            ps = mm_ps.tile([128, 512], f32, tag="mm", name="mm")
            box[0] = ps
            for fc in range(4):
                nc.tensor.matmul(
                    ps[:], xt[fc][:, tb * 128:(tb + 1) * 128], wv[fc][:],
                    start=(fc == 0), stop=False)

        def v_second(box, tb):
            ps = box[0]
            for fc in range(4, 8):
                nc.tensor.matmul(
                    ps[:], xt[fc][:, tb * 128:(tb + 1) * 128], wv[fc][:],
                    start=False, stop=(fc == 7))
            nc.vector.tensor_copy(
                VT[tb][:, :, 0:64],
                ps[:].rearrange("p (h d) -> p h d", d=64))
            nc.gpsimd.memset(VT[tb][:, :, 64:65], 1.0)

        def v_unit(tb):
            box = [None]
            v_first(box, tb)
            v_second(box, tb)

        def add_kq_filler(latest, dest, w, p, t):
            box = [None]
            add_filler(latest, lambda: kq_first(box, w, p, t))
            add_filler(latest, lambda: kq_second(box, dest, w, p, t))

        def add_v_filler(latest, tb):
            box = [None]
            add_filler(latest, lambda: v_first(box, tb))
            add_filler(latest, lambda: v_second(box, tb))

        # ---- fillers with deadlines ---------------------------------
        # each entry: (latest_gidx, closure); popped proportionally or
        # when the deadline is hit.
        fillers = []

        def add_filler(latest, fn):
            fillers.append((latest, fn))

        state = {"emitted": 0, "total": 0}

        def maybe_fill(done, steps):
            # pop through the last due entry (deadlines force front-running),
            # then keep pace proportionally.
            due_idx = -1
            for i, (latest, _) in enumerate(fillers):
                if latest is not None and done >= latest:
                    due_idx = i
            while due_idx >= 0:
                _, fn = fillers.pop(0)
                fn()
                state["emitted"] += 1
                due_idx -= 1
            while fillers:
                latest, fn = fillers[0]
                if state["emitted"] * steps >= done * state["total"]:
                    break
                fillers.pop(0)
                fn()
                state["emitted"] += 1

        # preamble: what head 0 / qc 0 needs up front
        kq_unit(KT, wk, 0, 0)
        kq_unit(KT, wk, 0, 1)
        kq_unit(QT, wq, 0, 0)
        kq_unit(QT, wq, 0, 1)
        for tb in range(8):
            v_unit(tb)

        # remaining proj work as deadline fillers. Deadlines are EMISSION
        # deadlines: S(g) is emitted at g-2, so a tile read at gidx g must
        # be written by done <= g-4 (margin for the in-body emission order).
        # Pair p's K/Q are spread evenly across pair p-1's gidx span to
        # avoid bursty dumps that starve ACT.
        for tb in range(8, 16):
            add_v_filler(max(0, tb - 4), tb)
        for t in range(2, 4):
            add_kq_filler(4 * t - 6, KT, wk, 0, t)
        for t in range(2, 4):
            add_kq_filler(12, QT, wq, 0, t)
        for p in range(1, NPAIR):
            base = 64 * (p - 1) + 8
            for i, (dest, w, t) in enumerate(
                    [(KT, wk, t) for t in range(4)] +
                    [(QT, wq, t) for t in range(4)]):
                add_kq_filler(base + 5 * i, dest, w, p, t)

        # ---- attention, software-pipelined across all 16 units ------
        units = [(p, hh, qc) for p in range(NPAIR) for hh in range(2)
                 for qc in range(2)]
        NU = len(units)
        GTOT = NU * 16

        def emit_S(gidx):
            u, kc = divmod(gidx, 16)
            p, hh, qc = units[u]
            hb = hh * 64
            st = s_ps.tile([128, 1024], f32, tag="s", name="s")
            for j in range(2):
                nc.tensor.matmul(
                    st[:, j * 512:(j + 1) * 512],
                    KT[p][hb:hb + 64, kc * 128:(kc + 1) * 128],
                    QT[p][hb:hb + 64,
                          qc * 1024 + j * 512:qc * 1024 + j * 512 + 512],
                    start=True, stop=True)
            return st

        def norm_rest(p, hh, qc, ur, dsb):
            """Lazy normalize: broadcast 1/D and multiply into UN."""
            hb = hh * 64
            for j in range(2):
                bc = mm_ps.tile([128, 512], f32, tag="mm", name="bc")
                nc.tensor.matmul(bc[:], ones_t[:],
                                 dsb[:, j * 512:(j + 1) * 512],
                                 start=True, stop=True)
                rs = r_pool.tile([64, 512], f32, tag="rs", name="rs")
                nc.vector.reciprocal_approx_fast(rs[:], bc[0:64, :])
                nc.gpsimd.tensor_mul(
                    UN[p][hb:hb + 64,
                          qc * 1024 + j * 512:qc * 1024 + j * 512 + 512],
                    ur[:, j * 512:(j + 1) * 512], rs[:])

        S_tiles = {0: emit_S(0), 1: emit_S(1)}
        U_box = [None]

        passA_added = [False]
        fin_state = {}

        def setup_fin():
            w_pool.release()
            xt_pool.release()
            fin_state["pool"] = tc.alloc_tile_pool(name="fin", bufs=1)
            fin_state["FIN"] = [
                fin_state["pool"].tile([128, DIM], f32, tag=f"fin{qf}",
                                       name=f"fin{qf}")
                for qf in range(16)]

        def passA(qf, of):
            FIN = fin_state["FIN"]
            ps = mm_ps.tile([128, 512], f32, tag="mm", name="pa")
            for p in range(3):
                nc.tensor.matmul(
                    ps[:], UN[p][:, qf * 128:(qf + 1) * 128],
                    WO[p][:, of * 512:(of + 1) * 512],
                    start=(p == 0), stop=(p == 2))
            nc.vector.tensor_add(
                FIN[qf][:, of * 512:(of + 1) * 512], ps[:],
                bias_t[:, of * 512:(of + 1) * 512])

        for gidx in range(GTOT):
            u, kc = divmod(gidx, 16)
            p, hh, qc = units[u]
            hloc = 2 * p + hh
            st = S_tiles.pop(gidx)
            es = es_pool.tile([128, 1024], bf16, tag="es", name="es")
            nc.scalar.activation(es[:], st[:], EXP, scale=0.125)
            # S lookahead FIRST: it feeds ACT(g+2), the critical chain;
            # PV(g) afterwards (it only feeds the U accumulation).
            if gidx + 2 < GTOT:
                S_tiles[gidx + 2] = emit_S(gidx + 2)
            if kc == 0:
                U_box[0] = u_ps.tile([65, 1024], f32, tag="u", name="u")
            U = U_box[0]
            for j in range(2):
                nc.tensor.matmul(
                    U[:, j * 512:(j + 1) * 512],
                    VT[kc][:, hloc, 0:65],
                    es[:, j * 512:(j + 1) * 512],
                    start=(kc == 0), stop=(kc == 15))
            if kc == 15:
                # fast U drain: free the single U psum buffer ASAP
                ur = ur_pool.tile([64, 1024], bf16, tag="ur", name="ur")
                nc.vector.tensor_copy(ur[:], U[0:64, :])
                dsb = d_pool.tile([1, 1024], f16, tag="d", name="d")
                with nc.allow_low_precision(reason="softmax denom f16"):
                    nc.vector.tensor_copy(dsb[:], U[64:65, :])
                add_filler(gidx + 24,
                           lambda p=p, hh=hh, qc=qc, ur=ur, dsb=dsb:
                           norm_rest(p, hh, qc, ur, dsb))
                if u == 11 and not passA_added[0]:
                    passA_added[0] = True
                    setup_fin()
                    for i, (qf, of) in enumerate(
                            (qf, of) for qf in range(16) for of in range(2)):
                        add_filler(gidx + 8 + i * 2,
                                   lambda qf=qf, of=of: passA(qf, of))
            state["total"] = max(state["total"], state["emitted"] + len(fillers))
            maybe_fill(gidx, GTOT)

        # flush remaining fillers (incl. last norms and any passA)
        while fillers:
            _, fn = fillers.pop(0)
            fn()

        # pass B: pair 3 onto FIN, then DMA out
        FIN = fin_state["FIN"]
        for qf in range(16):
            for of in range(2):
                ps = mm_ps.tile([128, 512], f32, tag="mm", name="pb")
                nc.tensor.matmul(
                    ps[:], UN[3][:, qf * 128:(qf + 1) * 128],
                    WO[3][:, of * 512:(of + 1) * 512],
                    start=True, stop=True)
                nc.vector.tensor_add(
                    FIN[qf][:, of * 512:(of + 1) * 512],
                    FIN[qf][:, of * 512:(of + 1) * 512], ps[:])
            nc.sync.dma_start(out_d.ap()[qf * 128:(qf + 1) * 128, :],
                              FIN[qf][:])

        fin_state["pool"].release()
        wo_pool.release()
        qkv_pool.release()

    nc.compile()
    return nc


def _get_nc():
    if "nc" not in _CACHE:
        _CACHE["nc"] = _build_nc()
    return _CACHE["nc"]


def _make_in_maps(x, w_qkv, w_out, b_out):
    bf = ml_dtypes.bfloat16
    xts = [np.ascontiguousarray(np.asarray(x[b], np.float32).T).astype(bf)
           for b in range(B)]
    wq_f = np.asarray(w_qkv[:, 0:1024], np.float32)
    wk_f = np.asarray(w_qkv[:, 1024:2048], np.float32)
    wv_f = np.asarray(w_qkv[:, 2048:3072], np.float32)
    wo_f = np.asarray(w_out, np.float32)  # [1024 inner, 1024 out]
    bias_rep = np.broadcast_to(
        np.asarray(b_out, np.float32).reshape(1, DIM), (128, DIM))
    zeros = np.zeros((128, DIM), np.float32)
    in_maps = []
    for i in range(NCORES):
        b, hh = i // 2, i % 2
        cs = slice(hh * 512, (hh + 1) * 512)
        wo_core = np.ascontiguousarray(
            wo_f[hh * 512:(hh + 1) * 512, :]).reshape(NPAIR, 128, DIM)
        in_maps.append({
            "xt": xts[b],
            "wq": np.ascontiguousarray(wq_f[:, cs]).astype(bf),
            "wk": np.ascontiguousarray(wk_f[:, cs]).astype(bf),
            "wv": np.ascontiguousarray(wv_f[:, cs]).astype(bf),
            "wo": wo_core.astype(bf),
            "bias": np.ascontiguousarray(
                (bias_rep if hh == 0 else zeros)).astype(bf),
        })
    return in_maps


def _assemble(results):
    out = np.empty((B, N, DIM), np.float32)
    for b in range(B):
        out[b] = results[2 * b]["out"] + results[2 * b + 1]["out"]
    return out


def run(x, w_qkv, w_out, b_out, trace=False):
    """Run the kernel; returns (output, BassKernelResults)."""
    from concourse.bass_utils import run_bass_kernel_spmd
    nc = _get_nc()
    in_maps = _make_in_maps(x, w_qkv, w_out, b_out)
    res = run_bass_kernel_spmd(nc, in_maps, core_ids=list(range(NCORES)),
                               trace=trace)
    return _assemble(res.results), res


def kernel(x, w_qkv, w_out, b_out):
    out, _ = run(x, w_qkv, w_out, b_out, trace=False)
    return out
